# revision 2
# baseline (speedup 1.0000x reference)
"""CharRNNEmbedding Trainium2 kernel v2: 2-layer biLSTM char encoder, 8 cores.

Data-parallel (512 words/core). Layer-1 collapsed to two single LSTM cells
(only h1[0,:,:H] and h1[-1,:,H:] are consumed; both are first-step outputs
from zero state).

v2 redesign vs baseline:
- h-projection in fp8e4m3 DoubleRow (K=256 in one 0.5-cyc/row matmul).
- x-path (char table, one-hot, W_ih) in bf16; gates pre-scaled by 32 (g-gate
  by 64) so activations apply scale=1/32 and tanh(g)=2*sigmoid(2g)-1.
- 3 ACT instructions per scan unit: sigmoid[i,f], sigmoid[o,g'], tanh(c).
- Elementwise in fp16 on DVE (stt-fused c update) + Pool (h writeback fp8).
- Embedding interleaved into the scan in end-first pair order (x15,x0 first).
- Zero-weight filler matmuls keep the PE p-state ramped while ACT-bound.
"""
import sys

sys.path.insert(0, "/opt/trn_rl_repo")

import copy as _copy
import numpy as np
import ml_dtypes
from contextlib import ExitStack

import concourse.bass as bass
import concourse.tile as tile
import concourse.mybir as mybir
from concourse import library_config
from concourse.bass_utils import run_bass_kernel_spmd

F32 = mybir.dt.float32
F32R = mybir.dt.float32r
F16 = mybir.dt.float16
BF16 = mybir.dt.bfloat16
FP8 = mybir.dt.float8e4
AF = mybir.ActivationFunctionType
ALU = mybir.AluOpType
DR = mybir.MatmulPerfMode.DoubleRow

NCORES = 8
B, S, T = 32, 128, 16
VOCAB, E, H = 262, 64, 256
NC_W = B * S // NCORES          # 512 words per core
TOK = NC_W * T
G4 = 4 * H                      # 1024 gate features per direction
WS = 32.0                       # gate pre-scale (sigma applies 1/32)

# wb8 (fp8) : whh doublerow packed [128, 2(dir), 2(half), 1024]
W8_COLS = 2 * 2 * G4
# w16 (bf16): wih0 aug [65, 2, 1024]
W16_COLS = 2 * G4
# wb32 (f32r): wih1 [128, 2, 5, 768] | wout [128, 5, 256]
OFF_WIH1 = 0
OFF_WOUT = OFF_WIH1 + 2 * 5 * 768
W32_COLS = OFF_WOUT + 5 * 256
# char table for indirect_copy gather: [128, 262] bf16; row 64 = 1.0 (bias
# row), rows 65..127 zero padding
CTBL_P = 128

# embedding pair order: pair e covers timesteps (e, 15-e); pair 0 runs
# upfront, pair e>=1 is emitted after scan group e-1
EMB_PAIRS = [(e, 15 - e) for e in range(8)]


def _pack_weights(inp):
    gate_scale = np.ones((G4,), np.float32) * WS
    gate_scale[2 * H:3 * H] = 2 * WS       # g-gate rows doubled

    wb8 = np.zeros((128, W8_COLS), ml_dtypes.float8_e4m3fn)
    w16 = np.zeros((128, W16_COLS), ml_dtypes.bfloat16)
    wb32 = np.zeros((128, W32_COLS), np.float32)

    for d, nm in enumerate("fb"):
        w = np.asarray(inp[f"w_ih_l0{nm}"], np.float32) * gate_scale[:, None]
        b = np.asarray(inp[f"b_l0{nm}"], np.float32) * gate_scale
        aug = np.concatenate([w.T, b[None, :]], 0)            # [65, 1024]
        w16[:65, d * G4:(d + 1) * G4] = aug.astype(ml_dtypes.bfloat16)
        whh = np.asarray(inp[f"w_hh_l0{nm}"], np.float32) * gate_scale[:, None]
        # doublerow: [p, half, m] = whh[m, half*128+p]
        wt = whh.T.reshape(2, 128, G4)                        # [half, p, m]
        dr = np.transpose(wt, (1, 0, 2)).reshape(128, 2 * G4)
        wb8[:, d * 2 * G4:(d + 1) * 2 * G4] = dr.astype(ml_dtypes.float8_e4m3fn)
        # layer 1 keeps gates i, o, g (f unused: c0 = 0); unscaled f32r
        w1 = np.asarray(inp[f"w_ih_l1{nm}"], np.float32)      # [1024, 512]
        b1 = np.asarray(inp[f"b_l1{nm}"], np.float32)
        sel = np.r_[0:256, 768:1024, 512:768]                 # i, o, g rows
        aug1 = np.concatenate([w1[sel].T, b1[sel][None, :]], 0)   # [513, 768]
        for k in range(5):
            lo, hi = k * 128, min((k + 1) * 128, 513)
            wb32[:hi - lo, OFF_WIH1 + (d * 5 + k) * 768:
                 OFF_WIH1 + (d * 5 + k + 1) * 768] = aug1[lo:hi]
    wo = np.asarray(inp["w_out"], np.float32)
    bo = np.asarray(inp["b_out"], np.float32)
    aug_o = np.concatenate([wo.T, bo[None, :]], 0)            # [513, 256]
    for k in range(5):
        lo, hi = k * 128, min((k + 1) * 128, 513)
        wb32[:hi - lo, OFF_WOUT + k * 256:OFF_WOUT + (k + 1) * 256] = aug_o[lo:hi]
    ce = np.asarray(inp["char_emb"], np.float32)
    ctbl = np.zeros((CTBL_P, VOCAB), np.float32)
    ctbl[:E] = ce.T                                           # [64, 262]
    ctbl[E] = 1.0                                             # bias row
    return ctbl.astype(ml_dtypes.bfloat16), w16, wb8, wb32


def _wrap_ids(ids_tm):
    """ids_tm [T, 512] int -> indirect_copy index layout [128, T, 32] uint16:
    idx j lives at [16g + j%16, t, j//16], replicated over the 8 groups."""
    w = np.empty((CTBL_P, T, NC_W // 16), np.uint16)
    wrapped = ids_tm.reshape(T, NC_W // 16, 16).transpose(2, 0, 1)  # [16,T,32]
    for g in range(CTBL_P // 16):
        w[16 * g:16 * (g + 1)] = wrapped
    return w


def _legalize_waits(nc, max_waits=1):
    """Split excess sync waits onto standalone no-ops. IndirectCopy cannot
    carry any sync wait in this walrus build."""
    ctr = 0
    for f in nc.m.functions:
        for blk in f.blocks:
            out = []
            for inst in blk.instructions:
                si = inst.sync_info
                mw = 0 if isinstance(inst, mybir.InstIndirectCopy) else max_waits
                if si is not None and si.on_wait and len(si.on_wait) > mw:
                    waits = list(si.on_wait)
                    keep = waits[len(waits) - mw:] if mw else []
                    extra = waits[:len(waits) - mw] if mw else waits
                    for w in extra:
                        nop = mybir.InstNoOp(name=f"I-wsplit-{ctr}")
                        ctr += 1
                        nop.engine = inst.engine
                        nop.sync_info = mybir.SyncInfo(on_wait=[w], on_update=[])
                        out.append(nop)
                    inst.sync_info = mybir.SyncInfo(
                        on_wait=keep, on_update=list(si.on_update))
                out.append(inst)
            blk.instructions = out
    return nc


OFFLOAD_TANH = False


def build_nc(n_warm=22, debug=False, n_filler=None):
    nc = bass.Bass()
    ctbl_d = nc.dram_tensor("ctbl", [CTBL_P, VOCAB], BF16, kind="ExternalInput")
    w16_d = nc.dram_tensor("w16", [128, W16_COLS], BF16, kind="ExternalInput")
    wb8_d = nc.dram_tensor("wb8", [128, W8_COLS], FP8, kind="ExternalInput")
    wb32_d = nc.dram_tensor("wb32", [128, W32_COLS], F32R, kind="ExternalInput")
    idsw_d = nc.dram_tensor("idsw", [CTBL_P, T, NC_W // 16], mybir.dt.uint16,
                            kind="ExternalInput")
    out_d = nc.dram_tensor("out", [128, 2, NC_W], F32, kind="ExternalOutput")
    dbg = {}
    if debug:
        for nm, shp, dt in (("dx2", [CTBL_P, T, NC_W], F32R),
                            ("dh8f", [128, 2, NC_W], FP8),
                            ("dc0", [128, 2, NC_W], F16),
                            ("dsnap", [128, 4, 2, NC_W], F32R),
                            ("dmerged", [128, 4, NC_W], F32R)):
            dbg[nm] = nc.dram_tensor(nm, shp, dt, kind="ExternalOutput")

    anchors_a = []        # first W1 x-matmul per unit (t>=1)
    anchors_b = []        # first W2 x-matmul per unit (t>=1)
    template = [None]     # filler template instruction handle

    with tile.TileContext(nc) as tc, ExitStack() as ctx:
        wpool = ctx.enter_context(tc.tile_pool(name="weights", bufs=1))
        spool = ctx.enter_context(tc.tile_pool(name="state", bufs=1))
        gpool = ctx.enter_context(tc.tile_pool(name="gates", bufs=2))
        epool = ctx.enter_context(tc.tile_pool(name="embed", bufs=3))
        psum = ctx.enter_context(tc.tile_pool(name="ps", bufs=2, space="PSUM"))

        # ---- weight loads, in order of first use (all on sync queue) ----
        ctbl = wpool.tile([CTBL_P, VOCAB], BF16)
        nc.sync.dma_start(ctbl[:], ctbl_d[:])
        idsw = wpool.tile([CTBL_P, T, NC_W // 16], mybir.dt.uint16)
        w16 = wpool.tile([128, W16_COLS], BF16)
        w32 = wpool.tile([128, W32_COLS], F32R)

        def ids_load(lo, hi):
            nc.sync.dma_start(idsw[:, lo:hi, :], idsw_d[:, lo:hi, :])

        ids_load(0, 1)
        nc.sync.dma_start(w16[:, 0:G4], w16_d[:, 0:G4])
        ids_load(15, 16)
        nc.sync.dma_start(w16[:, G4:2 * G4], w16_d[:, G4:2 * G4])
        ids_load(1, 2)
        ids_load(14, 15)
        w8 = wpool.tile([128, 2, 2, G4], FP8)
        nc.sync.dma_start(w8[:], wb8_d[:])
        ids_load(2, 8)
        ids_load(8, 14)
        nc.sync.dma_start(w32[:], wb32_d[:])

        def wih0(d):
            return w16[0:65, d * G4:(d + 1) * G4]

        def whh8(d):
            return w8[:, d, :, :]

        def wih1(d, k):
            o = OFF_WIH1 + (d * 5 + k) * 768
            return w32[:, o:o + 768]

        def wout(k):
            o = OFF_WOUT + k * 256
            return w32[:, o:o + 256]

        zf8 = wpool.tile([128, 2, NC_W], FP8)
        nc.gpsimd.memset(zf8[:], 0.0)
        ones = wpool.tile([128, 1, NC_W], F32R)
        ones_f = wpool.tile([128, 1, NC_W], F32)
        nc.vector.memset(ones_f[:], 0.0)
        nc.vector.memset(ones_f[0:1, :, :], 1.0)
        nc.vector.tensor_copy(ones[:], ones_f[:])

        # ---- state ----
        x2 = spool.tile([CTBL_P, T, NC_W], BF16)
        h8 = {d: spool.tile([128, 2, NC_W], FP8, name=f"h8_{d}") for d in range(2)}
        c = {d: spool.tile([128, 2, NC_W], F16, name=f"c_{d}") for d in range(2)}
        # f32r snapshots for layer 1: (0,0), (0,1), (15,0), (15,1)
        snap = {u: spool.tile([128, 2, NC_W], F32R, name=f"snap_{u[0]}_{u[1]}")
                for u in ((0, 0), (0, 1), (15, 0), (15, 1))}

        def embed_pair(e):
            for t in EMB_PAIRS[e]:
                nc.gpsimd.indirect_copy(x2[:, t, :], ctbl[:], idsw[:, t, :],
                                        i_know_ap_gather_is_preferred=True)

        CL = 3.4          # tanh(c): clamp, then Pade(3,2) t(27+t^2)/(27+9t^2)
        pending = []      # deferred tails

        def flush_tail():
            if not pending:
                return
            if not OFFLOAD_TANH:
                pt, pd, psog = pending.pop()
                tc_t = gpool.tile([128, 2, NC_W], F16, tag="tc",
                                  name=f"tc_{pt}_{pd}")
                nc.scalar.activation(tc_t[:], c[pd][:], AF.Tanh)
                nc.gpsimd.tensor_mul(h8[pd][:], psog[:, 0:2, :], tc_t[:])
                if (pt, pd) in snap:
                    nc.gpsimd.tensor_mul(snap[(pt, pd)][:], psog[:, 0:2, :],
                                         tc_t[:])
                return
            pt, pd, psog, ptcl, pn0, pr = pending.pop()
            n1 = gpool.tile([128, 2, NC_W], F16, tag="n1")
            nc.gpsimd.tensor_mul(n1[:], ptcl[:], pn0[:])
            m1 = gpool.tile([128, 2, NC_W], F16, tag="m1")
            nc.gpsimd.tensor_mul(m1[:], psog[:, 0:2, :], n1[:])
            nc.gpsimd.tensor_mul(h8[pd][:], m1[:], pr[:])
            if (pt, pd) in snap:
                nc.gpsimd.tensor_mul(snap[(pt, pd)][:], m1[:], pr[:])

        def scan_unit(t, d):
            xt = t if d == 0 else (T - 1 - t)
            # wave 1: banks [i0 i1 f0 f1]
            w1p = psum.tile([128, 4, NC_W], F32, tag="ps", name=f"W1_{t}_{d}")
            for pos, m in enumerate([0, 1, 2, 3]):
                hndl = nc.tensor.matmul(w1p[:, pos, :],
                                        wih0(d)[:, m * 128:(m + 1) * 128],
                                        x2[0:65, xt, :], start=True,
                                        stop=(t == 0))
                if pos == 0 and t > 0:
                    anchors_a.append(hndl.ins)
            if t > 0:
                for pos, m in enumerate([0, 1, 2, 3]):
                    nc.tensor.matmul(w1p[:, pos, :],
                                     whh8(d)[:, :, m * 128:(m + 1) * 128],
                                     h8[d][:], start=False, stop=True,
                                     perf_mode=DR)
            sig_if = gpool.tile([128, 4, NC_W], F16, tag="sif")
            nc.scalar.activation(sig_if[:], w1p[:], AF.Sigmoid, scale=1.0 / WS)
            u = w = si2 = None
            if OFFLOAD_TANH:
                if t > 0:
                    u = gpool.tile([128, 2, NC_W], F16, tag="u")
                    nc.gpsimd.tensor_mul(u[:], c[d][:], sig_if[:, 2:4, :])
            else:
                # A1-only work now so post-A2 leaves 2 ops before tanh(c):
                #   c' = c*sf + si*(2*sg-1) = (c*sf - si) + (2*si)*sg
                si2 = gpool.tile([128, 2, NC_W], F16, tag="si2")
                nc.vector.tensor_scalar(si2[:], sig_if[:, 0:2, :], 2.0, None,
                                        op0=ALU.mult)
                if t > 0:
                    u = gpool.tile([128, 2, NC_W], F16, tag="u")
                    nc.vector.tensor_mul(u[:], c[d][:], sig_if[:, 2:4, :])
                    w = gpool.tile([128, 2, NC_W], F16, tag="w")
                    nc.vector.tensor_sub(w[:], u[:], sig_if[:, 0:2, :])
            flush_tail()
            # wave 2: banks [o0 o1 g0 g1]
            w2p = psum.tile([128, 4, NC_W], F32, tag="ps", name=f"W2_{t}_{d}")
            for pos, m in enumerate([6, 7, 4, 5]):
                hndl = nc.tensor.matmul(w2p[:, pos, :],
                                        wih0(d)[:, m * 128:(m + 1) * 128],
                                        x2[0:65, xt, :], start=True,
                                        stop=(t == 0))
                if pos == 0 and t > 0:
                    anchors_b.append(hndl.ins)
                if pos == 0 and t == 1 and d == 0 and template[0] is None:
                    # filler template: accumulate zeros onto the open group
                    template[0] = nc.tensor.matmul(
                        w2p[0:16, 0, :], zf8[:, :, 0:16], zf8[:],
                        start=False, stop=False, perf_mode=DR,
                        skip_group_check=True)
            if t > 0:
                for pos, m in enumerate([6, 7, 4, 5]):
                    nc.tensor.matmul(w2p[:, pos, :],
                                     whh8(d)[:, :, m * 128:(m + 1) * 128],
                                     h8[d][:], start=False, stop=True,
                                     perf_mode=DR)
            sig_og = gpool.tile([128, 4, NC_W], F16, tag="sog", bufs=3)
            nc.scalar.activation(sig_og[:], w2p[:], AF.Sigmoid, scale=1.0 / WS)
            if not OFFLOAD_TANH:
                # post-A2 critical path: p = si2*sg ; c = w + p
                p = gpool.tile([128, 2, NC_W], F16, tag="tg")
                nc.vector.tensor_mul(p[:], si2[:], sig_og[:, 2:4, :])
                if t == 0:
                    nc.vector.tensor_sub(c[d][:], p[:], sig_if[:, 0:2, :])
                else:
                    nc.vector.tensor_add(c[d][:], w[:], p[:])
                pending.append((t, d, sig_og))
                return
            # c update (DVE): tg = 2*sg-1 ; t1 = si*tg ; c = u + t1
            tg = gpool.tile([128, 2, NC_W], F16, tag="tg")
            nc.vector.tensor_scalar(tg[:], sig_og[:, 2:4, :], 2.0, 1.0,
                                    op0=ALU.mult, op1=ALU.subtract)
            t1 = gpool.tile([128, 2, NC_W], F16, tag="t1")
            nc.vector.tensor_mul(t1[:], sig_if[:, 0:2, :], tg[:])
            if t == 0:
                nc.vector.tensor_copy(c[d][:], t1[:])
            else:
                nc.vector.tensor_add(c[d][:], u[:], t1[:])
            # tanh(c) via Pade on DVE/Pool; h = so * tanh(c) -> fp8
            tcl = gpool.tile([128, 2, NC_W], F16, tag="tcl")
            nc.vector.tensor_scalar(tcl[:], c[d][:], CL, -CL,
                                    op0=ALU.min, op1=ALU.max)
            u2 = gpool.tile([128, 2, NC_W], F16, tag="u2")
            nc.vector.tensor_mul(u2[:], tcl[:], tcl[:])
            n0 = gpool.tile([128, 2, NC_W], F16, tag="n0")
            nc.vector.tensor_scalar_add(n0[:], u2[:], 27.0)
            dn = gpool.tile([128, 2, NC_W], F16, tag="dn")
            nc.vector.tensor_scalar(dn[:], u2[:], 9.0, 27.0,
                                    op0=ALU.mult, op1=ALU.add)
            r = gpool.tile([128, 2, NC_W], F16, tag="r")
            with nc.allow_low_precision(reason="fp16 reciprocal (pade tanh)"):
                nc.vector.reciprocal(r[:], dn[:])
            pending.append((t, d, sig_og, tcl, n0, r))

        # ---- main schedule ----
        embed_pair(0)
        for t in range(T):
            for d in range(2):
                scan_unit(t, d)
            if 0 <= t <= 6:
                embed_pair(t + 1)
            if debug and t == 0:
                nc.sync.dma_start(dbg["dc0"][:], c[0][:])
        flush_tail()

        if debug:
            nc.sync.dma_start(dbg["dx2"][:], x2[:])
            nc.sync.dma_start(dbg["dh8f"][:], h8[0][:])
            for i, u in enumerate(((0, 0), (0, 1), (15, 0), (15, 1))):
                nc.sync.dma_start(dbg["dsnap"][:, i, :, :], snap[u][:])

        # ---- layer 1 (two single cells, f32r) + output projection ----
        merged = spool.tile([128, 4, NC_W], F32R)
        l1_rhs = {
            0: [snap[(0, 0)][:, 0, :], snap[(0, 0)][:, 1, :],
                snap[(15, 1)][:, 0, :], snap[(15, 1)][:, 1, :], ones[:, 0, :]],
            1: [snap[(15, 0)][:, 0, :], snap[(15, 0)][:, 1, :],
                snap[(0, 1)][:, 0, :], snap[(0, 1)][:, 1, :], ones[:, 0, :]],
        }
        KORD = [4, 0, 1, 2, 3]
        for d in (1, 0):      # d=1's inputs are ready one unit earlier
            g1A = psum.tile([128, 4, NC_W], F32, tag="ps", name=f"g1A_{d}")
            g1B = psum.tile([128, 2, NC_W], F32, tag="ps", name=f"g1B_{d}")
            for pos, m in enumerate([0, 1, 4, 5]):        # i0 i1 g0 g1
                for j, k in enumerate(KORD):
                    nc.tensor.matmul(g1A[:, pos, :],
                                     wih1(d, k)[:, m * 128:(m + 1) * 128],
                                     l1_rhs[d][k], start=(j == 0), stop=(j == 4))
            for pos, m in enumerate([2, 3]):              # o0 o1
                for j, k in enumerate(KORD):
                    nc.tensor.matmul(g1B[:, pos, :],
                                     wih1(d, k)[:, m * 128:(m + 1) * 128],
                                     l1_rhs[d][k], start=(j == 0), stop=(j == 4))
            sig1 = gpool.tile([128, 2, NC_W], F16, tag="sog", bufs=3)
            nc.scalar.activation(sig1[:], g1A[:, 0:2, :], AF.Sigmoid)
            tg1 = gpool.tile([128, 2, NC_W], F16, tag="tc")
            nc.scalar.activation(tg1[:], g1A[:, 2:4, :], AF.Tanh)
            so1 = gpool.tile([128, 2, NC_W], F16, tag="sif")
            nc.scalar.activation(so1[:], g1B[:], AF.Sigmoid)
            c1 = gpool.tile([128, 2, NC_W], F16, tag="v")
            nc.vector.tensor_mul(c1[:], sig1[:], tg1[:])
            tc1 = gpool.tile([128, 2, NC_W], F16, tag="u")
            nc.scalar.activation(tc1[:], c1[:], AF.Tanh)
            nc.gpsimd.tensor_mul(merged[:, d * 2:d * 2 + 2, :], so1[:], tc1[:])
        if debug:
            nc.sync.dma_start(dbg["dmerged"][:], merged[:])

        ob = spool.tile([128, 2, NC_W], F32)
        po = psum.tile([128, 2, NC_W], F32, tag="ps")
        mr = [merged[:, 0, :], merged[:, 1, :],
              merged[:, 2, :], merged[:, 3, :], ones[:, 0, :]]
        PKORD = [4, 2, 3, 0, 1]       # ones + bwd-cell (ready first) first
        for m in range(2):
            for j, k in enumerate(PKORD):
                nc.tensor.matmul(po[:, m, :], wout(k)[:, m * 128:(m + 1) * 128],
                                 mr[k], start=(j == 0), stop=(j == 4))
            nc.vector.tensor_copy(ob[:, m, :], po[:, m, :])
            eng = nc.sync if m == 0 else nc.scalar
            eng.dma_start(out_d[:, m, :], ob[:, m, :])

    # ---- warmup fillers: ramp the PE clock during the DMA lead-in ----
    if n_warm and template[0] is not None:
        tmpl_inst = template[0].ins
        ctr = 0
        for f in nc.m.functions:
            for blk in f.blocks:
                first_pe = next((i for i, inst in enumerate(blk.instructions)
                                 if isinstance(inst, mybir.InstMatmult)), None)
                if first_pe is None:
                    continue
                fills = []
                for _ in range(n_warm):
                    fi = _copy.copy(tmpl_inst)
                    fi.name = f"I-warm-{ctr}"
                    ctr += 1
                    fi.sync_info = mybir.SyncInfo(on_wait=[], on_update=[])
                    fills.append(fi)
                blk.instructions = (blk.instructions[:first_pe] + fills +
                                    blk.instructions[first_pe:])

    _legalize_waits(nc)
    return nc


_NC_CACHE = None


def kernel(**inputs):
    global _NC_CACHE
    if _NC_CACHE is None:
        _NC_CACHE = build_nc()
    nc = _NC_CACHE

    ctbl, w16, wb8, wb32 = _pack_weights(inputs)
    char_ids = np.asarray(inputs["char_ids"])
    in_maps = []
    for cc in range(NCORES):
        ids_c = char_ids.reshape(B * S, T)[cc * NC_W:(cc + 1) * NC_W]  # [512,16]
        idsw = _wrap_ids(np.ascontiguousarray(ids_c.T))
        in_maps.append({"ctbl": ctbl, "w16": w16, "wb8": wb8, "wb32": wb32,
                        "idsw": idsw})

    res = run_bass_kernel_spmd(nc, in_maps, list(range(NCORES)))

    outs = []
    for cc in range(NCORES):
        o = res.results[cc]["out"]                # [128, 2, 512]: feat = m*128+p
        outs.append(o.transpose(1, 0, 2).reshape(256, NC_W).T)
    full = np.concatenate(outs, 0)
    return full.reshape(B, S, H).astype(np.float32)


# revision 10
# speedup vs baseline: 1.0203x; 1.0203x over previous
"""CharRNNEmbedding Trainium2 kernel v2: 2-layer biLSTM char encoder, 8 cores.

Data-parallel (512 words/core). Layer-1 collapsed to two single LSTM cells
(only h1[0,:,:H] and h1[-1,:,H:] are consumed; both are first-step outputs
from zero state).

v2 redesign vs baseline:
- h-projection in fp8e4m3 DoubleRow (K=256 in one 0.5-cyc/row matmul).
- x-path (char table, one-hot, W_ih) in bf16; gates pre-scaled by 32 (g-gate
  by 64) so activations apply scale=1/32 and tanh(g)=2*sigmoid(2g)-1.
- 3 ACT instructions per scan unit: sigmoid[i,f], sigmoid[o,g'], tanh(c).
- Elementwise in fp16 on DVE (stt-fused c update) + Pool (h writeback fp8).
- Embedding interleaved into the scan in end-first pair order (x15,x0 first).
- Zero-weight filler matmuls keep the PE p-state ramped while ACT-bound.
"""
import sys

sys.path.insert(0, "/opt/trn_rl_repo")

import copy as _copy
import numpy as np
import ml_dtypes
from contextlib import ExitStack

import concourse.bass as bass
import concourse.tile as tile
import concourse.mybir as mybir
from concourse import library_config
from concourse.bass_utils import run_bass_kernel_spmd

F32 = mybir.dt.float32
F32R = mybir.dt.float32r
F16 = mybir.dt.float16
BF16 = mybir.dt.bfloat16
FP8 = mybir.dt.float8e4
AF = mybir.ActivationFunctionType
ALU = mybir.AluOpType
DR = mybir.MatmulPerfMode.DoubleRow

NCORES = 8
B, S, T = 32, 128, 16
VOCAB, E, H = 262, 64, 256
NC_W = B * S // NCORES          # 512 words per core
TOK = NC_W * T
G4 = 4 * H                      # 1024 gate features per direction
WS = 32.0                       # gate pre-scale (sigma applies 1/32)

# wb8 (fp8) : whh doublerow packed [128, 2(dir), 2(half), 1024]
W8_COLS = 2 * 2 * G4
# w16 (bf16): wih0 aug [65, 2, 1024]
W16_COLS = 2 * G4
# wb32 (f32r): wih1 [128, 2, 5, 768] | wout [128, 5, 256]
OFF_WIH1 = 0
OFF_WOUT = OFF_WIH1 + 2 * 5 * 768
W32_COLS = OFF_WOUT + 5 * 256
# char table for indirect_copy gather: [128, 262] bf16; row 64 = 1.0 (bias
# row), rows 65..127 zero padding
CTBL_P = 128

# embedding pair order: pair e covers timesteps (e, 15-e); pair 0 runs
# upfront, pair e>=1 is emitted after scan group e-1
EMB_PAIRS = [(e, 15 - e) for e in range(8)]


def _pack_weights(inp):
    gate_scale = np.ones((G4,), np.float32) * WS
    gate_scale[2 * H:3 * H] = 2 * WS       # g-gate rows doubled

    wb8 = np.zeros((128, W8_COLS), ml_dtypes.float8_e4m3fn)
    w16 = np.zeros((128, W16_COLS), ml_dtypes.bfloat16)
    wb32 = np.zeros((128, W32_COLS), np.float32)

    for d, nm in enumerate("fb"):
        w = np.asarray(inp[f"w_ih_l0{nm}"], np.float32) * gate_scale[:, None]
        b = np.asarray(inp[f"b_l0{nm}"], np.float32) * gate_scale
        aug = np.concatenate([w.T, b[None, :]], 0)            # [65, 1024]
        w16[:65, d * G4:(d + 1) * G4] = aug.astype(ml_dtypes.bfloat16)
        whh = np.asarray(inp[f"w_hh_l0{nm}"], np.float32) * gate_scale[:, None]
        # doublerow: [p, half, m] = whh[m, half*128+p]
        wt = whh.T.reshape(2, 128, G4)                        # [half, p, m]
        dr = np.transpose(wt, (1, 0, 2)).reshape(128, 2 * G4)
        wb8[:, d * 2 * G4:(d + 1) * 2 * G4] = dr.astype(ml_dtypes.float8_e4m3fn)
        # layer 1 keeps gates i, o, g (f unused: c0 = 0); unscaled f32r
        w1 = np.asarray(inp[f"w_ih_l1{nm}"], np.float32)      # [1024, 512]
        b1 = np.asarray(inp[f"b_l1{nm}"], np.float32)
        sel = np.r_[0:256, 768:1024, 512:768]                 # i, o, g rows
        aug1 = np.concatenate([w1[sel].T, b1[sel][None, :]], 0)   # [513, 768]
        for k in range(5):
            lo, hi = k * 128, min((k + 1) * 128, 513)
            wb32[:hi - lo, OFF_WIH1 + (d * 5 + k) * 768:
                 OFF_WIH1 + (d * 5 + k + 1) * 768] = aug1[lo:hi]
    wo = np.asarray(inp["w_out"], np.float32)
    bo = np.asarray(inp["b_out"], np.float32)
    aug_o = np.concatenate([wo.T, bo[None, :]], 0)            # [513, 256]
    for k in range(5):
        lo, hi = k * 128, min((k + 1) * 128, 513)
        wb32[:hi - lo, OFF_WOUT + k * 256:OFF_WOUT + (k + 1) * 256] = aug_o[lo:hi]
    ce = np.asarray(inp["char_emb"], np.float32)
    ctbl = np.zeros((CTBL_P, VOCAB), np.float32)
    ctbl[:E] = ce.T                                           # [64, 262]
    ctbl[E] = 1.0                                             # bias row
    return ctbl.astype(ml_dtypes.bfloat16), w16, wb8, wb32


def _wrap_ids(ids_tm):
    """ids_tm [T, 512] int -> indirect_copy index layout [128, T, 32] uint16:
    idx j lives at [16g + j%16, t, j//16], replicated over the 8 groups."""
    w = np.empty((CTBL_P, T, NC_W // 16), np.uint16)
    wrapped = ids_tm.reshape(T, NC_W // 16, 16).transpose(2, 0, 1)  # [16,T,32]
    for g in range(CTBL_P // 16):
        w[16 * g:16 * (g + 1)] = wrapped
    return w


def _legalize_waits(nc, max_waits=1):
    """Split excess sync waits onto standalone no-ops. IndirectCopy cannot
    carry any sync wait in this walrus build."""
    ctr = 0
    for f in nc.m.functions:
        for blk in f.blocks:
            out = []
            for inst in blk.instructions:
                si = inst.sync_info
                mw = 0 if isinstance(inst, mybir.InstIndirectCopy) else max_waits
                if si is not None and si.on_wait and len(si.on_wait) > mw:
                    waits = list(si.on_wait)
                    keep = waits[len(waits) - mw:] if mw else []
                    extra = waits[:len(waits) - mw] if mw else waits
                    for w in extra:
                        nop = mybir.InstNoOp(name=f"I-wsplit-{ctr}")
                        ctr += 1
                        nop.engine = inst.engine
                        nop.sync_info = mybir.SyncInfo(on_wait=[w], on_update=[])
                        out.append(nop)
                    inst.sync_info = mybir.SyncInfo(
                        on_wait=keep, on_update=list(si.on_update))
                out.append(inst)
            blk.instructions = out
    return nc


OFFLOAD_TANH = False  # False = tanh on ACT; "d1" = Pade offload for d=1 units


def build_nc(n_warm=22, debug=False, n_filler=None):
    nc = bass.Bass()
    ctbl_d = nc.dram_tensor("ctbl", [CTBL_P, VOCAB], BF16, kind="ExternalInput")
    w16_d = nc.dram_tensor("w16", [128, W16_COLS], BF16, kind="ExternalInput")
    wb8_d = nc.dram_tensor("wb8", [128, W8_COLS], FP8, kind="ExternalInput")
    wb32_d = nc.dram_tensor("wb32", [128, W32_COLS], F32R, kind="ExternalInput")
    idsw_d = nc.dram_tensor("idsw", [CTBL_P, T, NC_W // 16], mybir.dt.uint16,
                            kind="ExternalInput")
    out_d = nc.dram_tensor("out", [128, 2, NC_W], F32, kind="ExternalOutput")
    dbg = {}
    if debug:
        for nm, shp, dt in (("dx2", [CTBL_P, T, NC_W], F32R),
                            ("dh8f", [128, 2, NC_W], FP8),
                            ("dc0", [128, 2, NC_W], F16),
                            ("dsnap", [128, 4, 2, NC_W], F32R),
                            ("dmerged", [128, 4, NC_W], F32R)):
            dbg[nm] = nc.dram_tensor(nm, shp, dt, kind="ExternalOutput")

    anchors_a = []        # first W1 x-matmul per unit (t>=1)
    anchors_b = []        # first W2 x-matmul per unit (t>=1)
    template = [None]     # filler template instruction handle

    with tile.TileContext(nc) as tc, ExitStack() as ctx:
        wpool = ctx.enter_context(tc.tile_pool(name="weights", bufs=1))
        spool = ctx.enter_context(tc.tile_pool(name="state", bufs=1))
        gpool = ctx.enter_context(tc.tile_pool(name="gates", bufs=2))
        epool = ctx.enter_context(tc.tile_pool(name="embed", bufs=3))
        psum = ctx.enter_context(tc.tile_pool(name="ps", bufs=2, space="PSUM"))

        # ---- weight loads, in order of first use (all on sync queue) ----
        ctbl = wpool.tile([CTBL_P, VOCAB], BF16)
        nc.sync.dma_start(ctbl[:], ctbl_d[:])
        idsw = wpool.tile([CTBL_P, T, NC_W // 16], mybir.dt.uint16)
        w16 = wpool.tile([128, W16_COLS], BF16)
        w32 = wpool.tile([128, W32_COLS], F32R)

        def ids_load(lo, hi):
            nc.sync.dma_start(idsw[:, lo:hi, :], idsw_d[:, lo:hi, :])

        ids_load(0, 1)
        nc.sync.dma_start(w16[:, 0:G4], w16_d[:, 0:G4])
        ids_load(15, 16)
        nc.sync.dma_start(w16[:, G4:2 * G4], w16_d[:, G4:2 * G4])
        ids_load(1, 2)
        ids_load(14, 15)
        w8 = wpool.tile([128, 2, 2, G4], FP8)
        nc.sync.dma_start(w8[:], wb8_d[:])
        ids_load(2, 8)
        ids_load(8, 14)
        nc.sync.dma_start(w32[:], wb32_d[:])

        def wih0(d):
            return w16[0:65, d * G4:(d + 1) * G4]

        def whh8(d):
            return w8[:, d, :, :]

        def wih1(d, k):
            o = OFF_WIH1 + (d * 5 + k) * 768
            return w32[:, o:o + 768]

        def wout(k):
            o = OFF_WOUT + k * 256
            return w32[:, o:o + 256]

        zf8 = wpool.tile([128, 2, NC_W], FP8)
        nc.gpsimd.memset(zf8[:], 0.0)
        ones = wpool.tile([128, 1, NC_W], F32R)
        ones_f = wpool.tile([128, 1, NC_W], F32)
        nc.vector.memset(ones_f[:], 0.0)
        nc.vector.memset(ones_f[0:1, :, :], 1.0)
        nc.vector.tensor_copy(ones[:], ones_f[:])

        # ---- state ----
        x2 = spool.tile([CTBL_P, T, NC_W], BF16)
        h8 = {d: spool.tile([128, 2, NC_W], FP8, name=f"h8_{d}") for d in range(2)}
        c = {d: spool.tile([128, 2, NC_W], F16, name=f"c_{d}") for d in range(2)}
        # f32r snapshots for layer 1: (0,0), (0,1), (15,0), (15,1)
        snap = {u: spool.tile([128, 2, NC_W], F32R, name=f"snap_{u[0]}_{u[1]}")
                for u in ((0, 0), (0, 1), (15, 0), (15, 1))}

        def embed_pair(e):
            for t in EMB_PAIRS[e]:
                nc.gpsimd.indirect_copy(x2[:, t, :], ctbl[:], idsw[:, t, :],
                                        i_know_ap_gather_is_preferred=True)

        CL = 3.4          # tanh(c): clamp, then Pade(3,2) t(27+t^2)/(27+9t^2)
        pending = []      # deferred tails

        def flush_tail():
            if not pending:
                return
            e = pending.pop()
            if e[0] == "act":
                _, pt, pd, psog = e
                tc_t = gpool.tile([128, 2, NC_W], F16, tag="tc",
                                  name=f"tc_{pt}_{pd}")
                nc.scalar.activation(tc_t[:], c[pd][:], AF.Tanh)
                nc.gpsimd.tensor_mul(h8[pd][:], psog[:, 0:2, :], tc_t[:])
                if (pt, pd) in snap:
                    nc.gpsimd.tensor_mul(snap[(pt, pd)][:], psog[:, 0:2, :],
                                         tc_t[:])
                return
            _, pt, pd, psog, ptcl, pn0, pr = e
            n1 = gpool.tile([128, 2, NC_W], F16, tag="n1")
            nc.gpsimd.tensor_mul(n1[:], ptcl[:], pn0[:])
            m1 = gpool.tile([128, 2, NC_W], F16, tag="m1")
            nc.gpsimd.tensor_mul(m1[:], psog[:, 0:2, :], n1[:])
            nc.gpsimd.tensor_mul(h8[pd][:], m1[:], pr[:])
            if (pt, pd) in snap:
                nc.gpsimd.tensor_mul(snap[(pt, pd)][:], m1[:], pr[:])

        def scan_unit(t, d):
            xt = t if d == 0 else (T - 1 - t)
            # wave 1: banks [i0 i1 f0 f1]
            w1p = psum.tile([128, 4, NC_W], F32, tag="ps", name=f"W1_{t}_{d}")
            for pos, m in enumerate([0, 1, 2, 3]):
                hndl = nc.tensor.matmul(w1p[:, pos, :],
                                        wih0(d)[:, m * 128:(m + 1) * 128],
                                        x2[0:65, xt, :], start=True,
                                        stop=(t == 0))
                if pos == 0 and t > 0:
                    anchors_a.append(hndl.ins)
            if t > 0:
                for pos, m in enumerate([0, 1, 2, 3]):
                    nc.tensor.matmul(w1p[:, pos, :],
                                     whh8(d)[:, :, m * 128:(m + 1) * 128],
                                     h8[d][:], start=False, stop=True,
                                     perf_mode=DR)
            sig_if = gpool.tile([128, 4, NC_W], F16, tag="sif")
            nc.scalar.activation(sig_if[:], w1p[:], AF.Sigmoid, scale=1.0 / WS)
            offload = OFFLOAD_TANH is True or (OFFLOAD_TANH == "d1" and d == 1)
            # A1-only work now so post-A2 leaves 2 ops before tanh(c):
            #   c' = c*sf + si*(2*sg-1) = (c*sf - si) + (2*si)*sg
            eng_uw = nc.vector
            u = w = None
            si2 = gpool.tile([128, 2, NC_W], F16, tag="si2")
            nc.vector.tensor_scalar(si2[:], sig_if[:, 0:2, :], 2.0, None,
                                    op0=ALU.mult)
            if t > 0:
                u = gpool.tile([128, 2, NC_W], F16, tag="u")
                eng_uw.tensor_mul(u[:], c[d][:], sig_if[:, 2:4, :])
                w = gpool.tile([128, 2, NC_W], F16, tag="w")
                eng_uw.tensor_sub(w[:], u[:], sig_if[:, 0:2, :])
            flush_tail()
            # wave 2: banks [o0 o1 g0 g1]
            w2p = psum.tile([128, 4, NC_W], F32, tag="ps", name=f"W2_{t}_{d}")
            for pos, m in enumerate([6, 7, 4, 5]):
                hndl = nc.tensor.matmul(w2p[:, pos, :],
                                        wih0(d)[:, m * 128:(m + 1) * 128],
                                        x2[0:65, xt, :], start=True,
                                        stop=(t == 0))
                if pos == 0 and t > 0:
                    anchors_b.append(hndl.ins)
                if pos == 0 and t == 1 and d == 0 and template[0] is None:
                    # filler template: accumulate zeros onto the open group
                    template[0] = nc.tensor.matmul(
                        w2p[0:16, 0, :], zf8[:, :, 0:16], zf8[:],
                        start=False, stop=False, perf_mode=DR,
                        skip_group_check=True)
            if t > 0:
                for pos, m in enumerate([6, 7, 4, 5]):
                    nc.tensor.matmul(w2p[:, pos, :],
                                     whh8(d)[:, :, m * 128:(m + 1) * 128],
                                     h8[d][:], start=False, stop=True,
                                     perf_mode=DR)
            sig_og = gpool.tile([128, 4, NC_W], F16, tag="sog", bufs=3)
            nc.scalar.activation(sig_og[:], w2p[:], AF.Sigmoid, scale=1.0 / WS)
            # post-A2 critical path: p = si2*sg ; c = w + p
            p = gpool.tile([128, 2, NC_W], F16, tag="tg")
            nc.vector.tensor_mul(p[:], si2[:], sig_og[:, 2:4, :])
            if t == 0:
                nc.vector.tensor_sub(c[d][:], p[:], sig_if[:, 0:2, :])
            else:
                nc.vector.tensor_add(c[d][:], w[:], p[:])
            if not offload:
                pending.append(("act", t, d, sig_og))
                return
            # tanh(c) via Pade on DVE/Pool; h = so * tanh(c) -> fp8
            tcl = gpool.tile([128, 2, NC_W], F16, tag="tcl")
            nc.vector.tensor_scalar(tcl[:], c[d][:], CL, -CL,
                                    op0=ALU.min, op1=ALU.max)
            u2 = gpool.tile([128, 2, NC_W], F16, tag="u2")
            nc.vector.tensor_mul(u2[:], tcl[:], tcl[:])
            n0 = gpool.tile([128, 2, NC_W], F16, tag="n0")
            nc.vector.tensor_scalar_add(n0[:], u2[:], 27.0)
            dn = gpool.tile([128, 2, NC_W], F16, tag="dn")
            nc.vector.tensor_scalar(dn[:], u2[:], 9.0, 27.0,
                                    op0=ALU.mult, op1=ALU.add)
            r = gpool.tile([128, 2, NC_W], F16, tag="r")
            with nc.allow_low_precision(reason="fp16 reciprocal (pade tanh)"):
                nc.vector.reciprocal(r[:], dn[:])
            pending.append(("pade", t, d, sig_og, tcl, n0, r))

        # ---- main schedule ----
        embed_pair(0)
        for t in range(T):
            for d in range(2):
                scan_unit(t, d)
            if 0 <= t <= 6:
                embed_pair(t + 1)
            if debug and t == 0:
                nc.sync.dma_start(dbg["dc0"][:], c[0][:])
        flush_tail()

        if debug:
            nc.sync.dma_start(dbg["dx2"][:], x2[:])
            nc.sync.dma_start(dbg["dh8f"][:], h8[0][:])
            for i, u in enumerate(((0, 0), (0, 1), (15, 0), (15, 1))):
                nc.sync.dma_start(dbg["dsnap"][:, i, :, :], snap[u][:])

        # ---- layer 1 (two single cells, f32r) + output projection ----
        merged = spool.tile([128, 4, NC_W], F32R)
        l1_rhs = {
            0: [snap[(0, 0)][:, 0, :], snap[(0, 0)][:, 1, :],
                snap[(15, 1)][:, 0, :], snap[(15, 1)][:, 1, :], ones[:, 0, :]],
            1: [snap[(15, 0)][:, 0, :], snap[(15, 0)][:, 1, :],
                snap[(0, 1)][:, 0, :], snap[(0, 1)][:, 1, :], ones[:, 0, :]],
        }
        KORD = [4, 0, 1, 2, 3]
        for d in (1, 0):      # d=1's inputs are ready one unit earlier
            g1A = psum.tile([128, 4, NC_W], F32, tag="ps", name=f"g1A_{d}")
            g1B = psum.tile([128, 2, NC_W], F32, tag="ps", name=f"g1B_{d}")
            for pos, m in enumerate([0, 1, 4, 5]):        # i0 i1 g0 g1
                for j, k in enumerate(KORD):
                    nc.tensor.matmul(g1A[:, pos, :],
                                     wih1(d, k)[:, m * 128:(m + 1) * 128],
                                     l1_rhs[d][k], start=(j == 0), stop=(j == 4))
            for pos, m in enumerate([2, 3]):              # o0 o1
                for j, k in enumerate(KORD):
                    nc.tensor.matmul(g1B[:, pos, :],
                                     wih1(d, k)[:, m * 128:(m + 1) * 128],
                                     l1_rhs[d][k], start=(j == 0), stop=(j == 4))
            sig1 = gpool.tile([128, 2, NC_W], F16, tag="sog", bufs=3)
            nc.scalar.activation(sig1[:], g1A[:, 0:2, :], AF.Sigmoid)
            tg1 = gpool.tile([128, 2, NC_W], F16, tag="tc")
            nc.scalar.activation(tg1[:], g1A[:, 2:4, :], AF.Tanh)
            so1 = gpool.tile([128, 2, NC_W], F16, tag="sif")
            nc.scalar.activation(so1[:], g1B[:], AF.Sigmoid)
            c1 = gpool.tile([128, 2, NC_W], F16, tag="v")
            nc.vector.tensor_mul(c1[:], sig1[:], tg1[:])
            tc1 = gpool.tile([128, 2, NC_W], F16, tag="u")
            nc.scalar.activation(tc1[:], c1[:], AF.Tanh)
            nc.gpsimd.tensor_mul(merged[:, d * 2:d * 2 + 2, :], so1[:], tc1[:])
        if debug:
            nc.sync.dma_start(dbg["dmerged"][:], merged[:])

        ob = spool.tile([128, 2, NC_W], F32)
        po = psum.tile([128, 2, NC_W], F32, tag="ps")
        mr = [merged[:, 0, :], merged[:, 1, :],
              merged[:, 2, :], merged[:, 3, :], ones[:, 0, :]]
        PKORD = [4, 2, 3, 0, 1]       # ones + bwd-cell (ready first) first
        for m in range(2):
            for j, k in enumerate(PKORD):
                nc.tensor.matmul(po[:, m, :], wout(k)[:, m * 128:(m + 1) * 128],
                                 mr[k], start=(j == 0), stop=(j == 4))
            nc.vector.tensor_copy(ob[:, m, :], po[:, m, :])
            eng = nc.sync if m == 0 else nc.scalar
            eng.dma_start(out_d[:, m, :], ob[:, m, :])

    # ---- warmup fillers: ramp the PE clock during the DMA lead-in.
    # Insert before the first Ldweights/Matmult so the lead DMA wait on the
    # first Ldweights doesn't head-of-line-block them. ----
    if n_warm and template[0] is not None:
        tmpl_inst = template[0].ins
        ctr = 0
        for f in nc.m.functions:
            for blk in f.blocks:
                first_pe = next(
                    (i for i, inst in enumerate(blk.instructions)
                     if isinstance(inst, (mybir.InstMatmult,
                                          mybir.InstLdweights))), None)
                if first_pe is None:
                    continue
                fills = []
                for _ in range(n_warm):
                    fi = _copy.copy(tmpl_inst)
                    fi.name = f"I-warm-{ctr}"
                    ctr += 1
                    fi.sync_info = mybir.SyncInfo(on_wait=[], on_update=[])
                    fills.append(fi)
                blk.instructions = (blk.instructions[:first_pe] + fills +
                                    blk.instructions[first_pe:])

    _legalize_waits(nc)
    return nc


_NC_CACHE = None


def kernel(**inputs):
    global _NC_CACHE
    if _NC_CACHE is None:
        _NC_CACHE = build_nc()
    nc = _NC_CACHE

    ctbl, w16, wb8, wb32 = _pack_weights(inputs)
    char_ids = np.asarray(inputs["char_ids"])
    in_maps = []
    for cc in range(NCORES):
        ids_c = char_ids.reshape(B * S, T)[cc * NC_W:(cc + 1) * NC_W]  # [512,16]
        idsw = _wrap_ids(np.ascontiguousarray(ids_c.T))
        in_maps.append({"ctbl": ctbl, "w16": w16, "wb8": wb8, "wb32": wb32,
                        "idsw": idsw})

    res = run_bass_kernel_spmd(nc, in_maps, list(range(NCORES)))

    outs = []
    for cc in range(NCORES):
        o = res.results[cc]["out"]                # [128, 2, 512]: feat = m*128+p
        outs.append(o.transpose(1, 0, 2).reshape(256, NC_W).T)
    full = np.concatenate(outs, 0)
    return full.reshape(B, S, H).astype(np.float32)
